# revision 1
# baseline (speedup 1.0000x reference)
"""Distributed 3-layer GAT + global mean pool + linear head on 8 Trainium2
NeuronCores (Bass/Tile, SPMD via run_bass_kernel_spmd).

Strategy: 1D node partitioning by destination. Each core owns 6272 table rows
(49 blocks x 128; 50176 >= 50000, device 7 padded). Per layer:
  dense:  table_slice[6272, EW] = xT_own @ Waug  (fp32r matmuls)
          (table row = [h | al_src | pad], aldst kept in a local table)
  AG:     AllGather table slices -> full table (row id = permuted node id)
  edge:   bulk dma_gather of source rows (int16 idx, 2 src-range buckets),
          per-edge aldst gather from the local table, attention
          ex = exp(leakyrelu(alsrc+aldst)) folded into the gathered rows,
          segment-sum via one-hot selector matmuls into PSUM [dst, 256+H],
          finalize: /denom + bias + ELU -> transposed into SBUF for the next
          dense phase (L3: pooled via one-hot graph selector matmuls).
Pool partials AllReduced, scaled by 1/cnt, final linear -> [64, 10].
"""
import numpy as np

import concourse.bass as bass
import concourse.bacc as bacc
import concourse.tile as tile
from concourse import mybir
from concourse import bass_utils

EDGE_MODE = "full"   # "gather" | "compute" | "full" (bisection aid)

f32 = mybir.dt.float32
f32r = mybir.dt.float32r
i16 = mybir.dt.int16

# problem constants (hardcoded per contract)
N, IN, HID, HEADS, OUT, NG = 50000, 128, 64, 4, 10, 64
E = 600000
P = 128
NCORES = 8
NB = 49                      # blocks per core
NODES_PER = NB * P           # 6272 rows per core
NTOT = NODES_PER * NCORES    # 50176
R4 = 4 * NODES_PER           # 25088 bucket split (rows of cores 0-3)
GROUP_BLOCKS = 2
EW12 = 320                   # table row width (f32) layers 1/2; 1280B, %256
EWM12 = 264                  # meaningful cols: 256 h + 4 alsrc + 4 aldst
EW3 = 128                    # table row width layer 3; 512B
EWM3 = 68                    # 64 h + 1 alsrc + 1 aldst (+2 pad for fp32r %4)
HC = HEADS * HID             # 256


# ----------------------------------------------------------------------------
# host-side graph preprocessing
# ----------------------------------------------------------------------------

def _assign_nodes(src, dst):
    """node -> row permutation balancing per-(core,block,bucket) edge counts.

    Phase A: nodes -> cores balancing total in-degree.
    Phase B: within each core, nodes -> blocks balancing (lo,hi) in-degree.
    Returns perm[node] = global row id in [0, NTOT).
    """
    degin = np.bincount(dst, minlength=N).astype(np.int64)
    order = np.argsort(-degin, kind="stable")
    # phase A: greedy over 8 cores
    core_load = np.zeros(NCORES, dtype=np.int64)
    core_cnt = np.zeros(NCORES, dtype=np.int64)
    node_core = np.empty(N, dtype=np.int64)
    for n in order:
        c = np.argmin(np.where(core_cnt < NODES_PER, core_load, np.iinfo(np.int64).max))
        node_core[n] = c
        core_load[c] += degin[n]
        core_cnt[c] += 1
    # bucket degree per dst node (src in cores 0-3 vs 4-7)
    lo_mask = node_core[src] < 4
    deg_lo = np.bincount(dst[lo_mask], minlength=N).astype(np.int64)
    deg_hi = degin - deg_lo
    # phase B: per core, nodes -> blocks
    perm = np.empty(N, dtype=np.int64)
    for c in range(NCORES):
        nodes = np.where(node_core == c)[0]
        nd = nodes[np.argsort(-(deg_lo[nodes] + deg_hi[nodes]), kind="stable")]
        blo = np.zeros(NB, dtype=np.int64)
        bhi = np.zeros(NB, dtype=np.int64)
        bcnt = np.zeros(NB, dtype=np.int64)
        slot = np.zeros(NB, dtype=np.int64)
        for n in nd:
            b = np.argmin(np.where(bcnt < P, blo + bhi, np.iinfo(np.int64).max))
            perm[n] = c * NODES_PER + b * P + slot[b]
            blo[b] += deg_lo[n]
            bhi[b] += deg_hi[n]
            bcnt[b] += 1
            slot[b] += 1
    return perm


def _build_schedule(srcrow, dstrow):
    """Shape-uniform SPMD edge schedule.

    Returns dict with per-core arrays (hidx, aidx, dloc) and the shared
    group/tile structure.
    """
    core = dstrow // NODES_PER
    blk = (dstrow % NODES_PER) // P
    dloc_all = (dstrow % NODES_PER) % P
    bkt = (srcrow >= R4).astype(np.int64)

    # edge lists per (core, block, bucket)
    lists = [[[None, None] for _ in range(NB)] for _ in range(NCORES)]
    for c in range(NCORES):
        mc = core == c
        for k in (0, 1):
            mk = mc & (bkt == k)
            sb = srcrow[mk]
            db = blk[mk]
            dl = dloc_all[mk]
            o = np.argsort(db, kind="stable")
            sb, db, dl = sb[o], db[o], dl[o]
            cuts = np.searchsorted(db, np.arange(NB + 1))
            for b in range(NB):
                lists[c][b][k] = (sb[cuts[b]:cuts[b + 1]], dl[cuts[b]:cuts[b + 1]])

    T = np.zeros((NB, 2), dtype=np.int64)
    for b in range(NB):
        for k in (0, 1):
            mx = max(len(lists[c][b][k][0]) for c in range(NCORES))
            T[b, k] = (mx + P - 1) // P

    groups = [list(range(g0, min(g0 + GROUP_BLOCKS, NB)))
              for g0 in range(0, NB, GROUP_BLOCKS)]

    # stream: per group: for k in (0,1): for b in group: T[b,k] tiles
    tile_block = []      # block id per stream tile
    group_info = []      # per group: (tile_offset, n_lo, n_hi, blocks)
    toff = 0
    for blocks in groups:
        n_lo = int(sum(T[b, 0] for b in blocks)) * P
        n_hi = int(sum(T[b, 1] for b in blocks)) * P
        for k in (0, 1):
            for b in blocks:
                tile_block.extend([b] * int(T[b, k]))
        group_info.append((toff, n_lo, n_hi, blocks))
        toff += (n_lo + n_hi) // P
    ntiles = toff

    hidx = np.zeros((NCORES, ntiles * P), dtype=np.int16)
    aidx = np.zeros((NCORES, ntiles * P), dtype=np.int16)
    dloc = np.full((NCORES, ntiles * P), -1.0, dtype=np.float32)
    for c in range(NCORES):
        pos = 0
        for blocks in groups:
            for k in (0, 1):
                for b in blocks:
                    s, dl = lists[c][b][k]
                    n = len(s)
                    npad = int(T[b, k]) * P
                    hidx[c, pos:pos + n] = (s - k * R4).astype(np.int16)
                    aidx[c, pos:pos + n] = (b * P + dl).astype(np.int16)
                    dloc[c, pos:pos + n] = dl.astype(np.float32)
                    pos += npad
        assert pos == ntiles * P

    def wrap16(a):
        # logical i -> [16, n/16] col-major-ish: (p = i%16, s = i//16),
        # replicated to 128 partitions
        m = a.reshape(-1, 16).T
        return np.tile(m, (8, 1)).copy()

    return {
        "hidx": np.stack([wrap16(hidx[c]) for c in range(NCORES)]),
        "aidx": np.stack([wrap16(aidx[c]) for c in range(NCORES)]),
        "dloc": np.stack([dloc[c].reshape(ntiles, P).T.copy()
                          for c in range(NCORES)]),   # [128, ntiles]
        "tile_block": tile_block,
        "group_info": group_info,
        "ntiles": ntiles,
    }


def _augment_weights(W, asrc, adst, ew):
    """[W | W@Asrc | W@Adst] padded to width ew."""
    IN_, HCl = W.shape
    H = asrc.shape[0]
    C = HCl // H
    A_s = np.zeros((HCl, H), np.float32)
    A_d = np.zeros((HCl, H), np.float32)
    for hd in range(H):
        A_s[hd * C:(hd + 1) * C, hd] = asrc[hd]
        A_d[hd * C:(hd + 1) * C, hd] = adst[hd]
    out = np.zeros((IN_, ew), np.float32)
    out[:, :HCl] = W
    out[:, HCl:HCl + H] = W @ A_s
    out[:, HCl + H:HCl + 2 * H] = W @ A_d
    return out


# ----------------------------------------------------------------------------
# device program
# ----------------------------------------------------------------------------

def _build_program(sched, time_reps=1,
                   stages=("d1", "e1", "d2", "e2", "d3", "e3", "pool")):
    nc = bacc.Bacc("TRN2", target_bir_lowering=False, debug=False,
                   enable_asserts=False, num_devices=NCORES)
    ntiles = sched["ntiles"]
    group_info = sched["group_info"]
    tile_block = sched["tile_block"]
    tgmax = max((nl + nh) // P for _, nl, nh, _ in group_info)

    # ---- kernel I/O ----
    t_xT = nc.dram_tensor("xT", [P, NODES_PER], f32r, kind="ExternalInput")
    t_w1 = nc.dram_tensor("W1a", [P, EWM12], f32r, kind="ExternalInput")
    t_w2 = nc.dram_tensor("W2a", [P, 2, EWM12], f32r, kind="ExternalInput")
    t_w3 = nc.dram_tensor("W3a", [P, 2, EWM3], f32r, kind="ExternalInput")
    t_b1 = nc.dram_tensor("BIAS1", [P, HC], f32, kind="ExternalInput")
    t_b2 = nc.dram_tensor("BIAS2", [P, HC], f32, kind="ExternalInput")
    t_b3 = nc.dram_tensor("BIAS3", [P, HID], f32, kind="ExternalInput")
    t_iota = nc.dram_tensor("IOTA2", [P, 2 * P], f32, kind="ExternalInput")
    t_ident = nc.dram_tensor("IDENT", [P, P], f32, kind="ExternalInput")
    t_hidx = nc.dram_tensor("HIDX", [P, ntiles * 8], i16, kind="ExternalInput")
    t_aidx = nc.dram_tensor("AIDX", [P, ntiles * 8], i16, kind="ExternalInput")
    t_dloc = nc.dram_tensor("DLOC", [P, ntiles], f32, kind="ExternalInput")
    t_bat = nc.dram_tensor("BATCH", [P, NB], f32, kind="ExternalInput")
    t_icnt = nc.dram_tensor("INVCNT", [NG, 1], f32, kind="ExternalInput")
    t_linw = nc.dram_tensor("LINW", [P, OUT], f32r, kind="ExternalInput")
    t_linb = nc.dram_tensor("LINB", [NG, OUT], f32, kind="ExternalInput")
    t_out = nc.dram_tensor("out", [NG, OUT], f32, kind="ExternalOutput")
    t_dbg1 = t_dbg2 = None
    if "pool" not in stages:
        t_dbg1 = nc.dram_tensor("dbg1", [P, EWM12], f32, kind="ExternalOutput")
        t_dbg2 = nc.dram_tensor("dbg2", [P, 2 * P], f32, kind="ExternalOutput")

    layers = [
        dict(ew=EW12, ewm=EWM12, hc=HC, h=HEADS, nchunk=1, rwm=260),
        dict(ew=EW12, ewm=EWM12, hc=HC, h=HEADS, nchunk=2, rwm=260),
        dict(ew=EW3, ewm=EWM3, hc=HID, h=1, nchunk=2, rwm=68),
    ]

    with tile.TileContext(nc) as tc:
        with tc.tile_pool(name="const", bufs=1) as cpool, \
             tc.tile_pool(name="xT", bufs=1) as xpool, \
             tc.tile_pool(name="gat", bufs=2) as gpool, \
             tc.tile_pool(name="work", bufs=4) as wpool, \
             tc.tile_pool(name="fin", bufs=2) as fpool, \
             tc.tile_pool(name="psacc", bufs=4, space="PSUM") as psacc, \
             tc.tile_pool(name="psmisc", bufs=2, space="PSUM") as psmisc, \
             tc.tile_pool(name="pspool", bufs=1, space="PSUM") as pspool, \
             tc.tile_pool(name="dram", bufs=1, space="DRAM") as dpool:

            # ---- consts into SBUF ----
            iota2 = cpool.tile([P, 2 * P], f32)
            nc.sync.dma_start(iota2[:], t_iota.ap())
            ident = cpool.tile([P, P], f32)
            nc.sync.dma_start(ident[:], t_ident.ap())
            biases = []
            for tb, w in ((t_b1, HC), (t_b2, HC), (t_b3, HID)):
                bt = cpool.tile([P, w], f32, tag=f"bias{len(biases)}")
                nc.sync.dma_start(bt[:], tb.ap())
                biases.append(bt)
            w1t = cpool.tile([P, 1, EWM12], f32r, tag="w1")
            nc.sync.dma_start(w1t[:, 0, :], t_w1.ap())
            w2t = cpool.tile([P, 2, EWM12], f32r, tag="w2")
            nc.sync.dma_start(w2t[:], t_w2.ap())
            w3t = cpool.tile([P, 2, EWM3], f32r, tag="w3")
            nc.sync.dma_start(w3t[:], t_w3.ap())
            w_sb = [w1t, w2t, w3t]
            batc = cpool.tile([P, NB], f32)
            nc.sync.dma_start(batc[:], t_bat.ap())
            icnt = cpool.tile([NG, 1], f32)
            nc.sync.dma_start(icnt[:], t_icnt.ap())
            linw = cpool.tile([P, OUT], f32r)
            nc.sync.dma_start(linw[:], t_linw.ap())
            linb = cpool.tile([NG, OUT], f32)
            nc.sync.dma_start(linb[:], t_linb.ap())

            # persistent transposed activations [P, 2, NODES_PER]
            xT = xpool.tile([P, 2, NODES_PER], f32r)
            nc.sync.dma_start(xT[:, 0, :], t_xT.ap())

            # per-layer DRAM scratch
            tslice = [dpool.tile([NODES_PER, EW12], f32, tag="ts0", name="ts0"),
                      dpool.tile([NODES_PER, EW12], f32, tag="ts1", name="ts1"),
                      dpool.tile([NODES_PER, EW3], f32, tag="ts2", name="ts2")]
            tfull = [dpool.tile([NTOT, EW12], f32, tag="tf0", name="tf0"),
                     dpool.tile([NTOT, EW12], f32, tag="tf1", name="tf1"),
                     dpool.tile([NTOT, EW3], f32, tag="tf2", name="tf2")]
            ald = [dpool.tile([NODES_PER, 64], f32, tag="al0", name="al0"),
                   dpool.tile([NODES_PER, 64], f32, tag="al1", name="al1"),
                   dpool.tile([NODES_PER, 64], f32, tag="al2", name="al2")]
            pool_in = dpool.tile([NG, HID], f32, tag="pin")
            pool_out = dpool.tile([NG, HID], f32, tag="pout")

            pool_ps = pspool.tile([NG, HID], f32, space="PSUM")

            def dense_phase(L):
                lay = layers[L]
                ew, ewm, nchunk = lay["ew"], lay["ewm"], lay["nchunk"]
                for m in range(NB):
                    ps = psmisc.tile([P, ewm], f32, space="PSUM", tag="ms", name="dps")
                    for c in range(nchunk):
                        nc.tensor.matmul(
                            ps[:], xT[:, c, m * P:(m + 1) * P], w_sb[L][:, c, :],
                            start=(c == 0), stop=(c == nchunk - 1))
                    dsl = wpool.tile([P, ewm], f32, tag="dsl")
                    nc.vector.tensor_copy(dsl[:], ps[:])
                    hc, h = lay["hc"], lay["h"]
                    nc.sync.dma_start(
                        tslice[L][m * P:(m + 1) * P, 0:hc + h], dsl[:, 0:hc + h])
                    nc.sync.dma_start(
                        ald[L][m * P:(m + 1) * P, 0:h], dsl[:, hc + h:hc + 2 * h])

            def edge_phase(L):
                lay = layers[L]
                ew, hc, h = lay["ew"], lay["hc"], lay["h"]
                rw = lay["rwm"]                   # matmul rhs width (%4)
                rpad = rw
                ti = 0                            # global stream tile idx
                for (toff, n_lo, n_hi, blocks) in group_info:
                    tg = (n_lo + n_hi) // P
                    G = gpool.tile([P, tgmax, ew], f32, tag="G")
                    ALD = gpool.tile([P, tgmax, 64], f32, tag="ALD")
                    hix = gpool.tile([P, tgmax * 8], i16, tag="hix")
                    aix = gpool.tile([P, tgmax * 8], i16, tag="aix")
                    dlc = gpool.tile([P, tgmax], f32, tag="dlc")
                    o16 = toff * 8
                    nc.sync.dma_start(hix[:, 0:tg * 8],
                                      t_hidx.ap()[:, o16:o16 + tg * 8])
                    nc.sync.dma_start(aix[:, 0:tg * 8],
                                      t_aidx.ap()[:, o16:o16 + tg * 8])
                    nc.sync.dma_start(dlc[:, 0:tg], t_dloc.ap()[:, toff:toff + tg])
                    if n_lo:
                        nc.gpsimd.dma_gather(
                            G[:, 0:n_lo // P, :], tfull[L][0:R4, :],
                            hix[:, 0:n_lo // 16], num_idxs=n_lo,
                            num_idxs_reg=n_lo, elem_size=ew, elem_step=ew,
                            single_packet=False)
                    if n_hi:
                        nc.gpsimd.dma_gather(
                            G[:, n_lo // P:tg, :], tfull[L][R4:NTOT, :],
                            hix[:, n_lo // 16:tg * 8], num_idxs=n_hi,
                            num_idxs_reg=n_hi, elem_size=ew, elem_step=ew,
                            single_packet=False)
                    nc.gpsimd.dma_gather(
                        ALD[:, 0:tg, :], ald[L][:, :], aix[:, 0:tg * 8],
                        num_idxs=tg * P, num_idxs_reg=tg * P,
                        elem_size=64, elem_step=64, single_packet=False)

                    if EDGE_MODE == "gather":
                        junk = wpool.tile([P, 8], f32, tag="junk")
                        nc.vector.tensor_tensor(
                            junk[:, 0:4], G[:, 0, 0:4], ALD[:, 0, 0:4],
                            mybir.AluOpType.add)
                        ti += tg
                        continue

                    # PSUM accumulators for this group's blocks
                    accs = {b: psacc.tile([P, rw], f32, space="PSUM", tag="acc",
                                          name=f"acc{b}")
                            for b in blocks}
                    started = {b: False for b in blocks}
                    remaining = {b: tile_block[toff:toff + tg].count(b)
                                 for b in blocks}

                    # process tiles in pairs
                    j = 0
                    while j < tg:
                        w2 = 2 if j + 1 < tg else 1
                        S = wpool.tile([P, 2, P], f32r, tag="S")
                        nc.vector.tensor_tensor(
                            S[:, 0:w2, :],
                            dlc[:, j:j + w2].unsqueeze(-1).to_broadcast([P, w2, P]),
                            iota2[:].rearrange("p (t i) -> p t i", t=2)[:, 0:w2, :],
                            mybir.AluOpType.is_equal)
                        et = wpool.tile([P, 2, h], f32, tag="et")
                        nc.vector.tensor_tensor(
                            et[:, 0:w2, :],
                            G[:, j:j + w2, hc:hc + h],
                            ALD[:, j:j + w2, 0:h], mybir.AluOpType.add)
                        lr = wpool.tile([P, 2, h], f32, tag="lr")
                        nc.vector.scalar_tensor_tensor(
                            lr[:, 0:w2, :], et[:, 0:w2, :], 0.2, et[:, 0:w2, :],
                            mybir.AluOpType.mult, mybir.AluOpType.max)
                        ex = wpool.tile([P, 2, h], f32, tag="ex")
                        nc.scalar.activation(ex[:, 0:w2, :], lr[:, 0:w2, :],
                                             mybir.ActivationFunctionType.Exp)
                        R = wpool.tile([P, 2, rpad], f32r, tag="R")
                        nc.vector.tensor_tensor(
                            R[:, 0:w2, 0:hc].rearrange(
                                "p t (hh c) -> p t hh c", hh=h),
                            G[:, j:j + w2, 0:hc].rearrange(
                                "p t (hh c) -> p t hh c", hh=h),
                            ex[:, 0:w2, :].unsqueeze(-1).to_broadcast(
                                [P, w2, h, HID]),
                            mybir.AluOpType.mult)
                        nc.scalar.activation(R[:, 0:w2, hc:hc + h], ex[:, 0:w2, :],
                                             mybir.ActivationFunctionType.Copy)
                        for q in range(w2):
                            b = tile_block[ti + j + q]
                            nc.tensor.matmul(
                                accs[b][:], S[:, q, :], R[:, q, 0:rw],
                                start=not started[b], stop=remaining[b] == 1,
                                skip_group_check=True)
                            started[b] = True
                            remaining[b] -= 1
                        j += w2

                    # finalize blocks of this group
                    if EDGE_MODE == "compute":
                        for b in blocks:
                            junk2 = wpool.tile([P, 8], f32, tag="junk2")
                            nc.vector.tensor_copy(junk2[:, 0:4],
                                                  accs[b][:, 0:4])
                        ti += tg
                        continue
                    for b in blocks:
                        acc = accs[b]
                        den = wpool.tile([P, h], f32, tag="den")
                        nc.vector.tensor_scalar_max(den[:], acc[:, hc:hc + h],
                                                    1e-30)
                        rec = wpool.tile([P, h], f32, tag="rec")
                        nc.vector.reciprocal(rec[:], den[:])
                        xb = fpool.tile([P, hc], f32, tag="xb")
                        nc.vector.tensor_tensor(
                            xb[:].rearrange("p (hh c) -> p hh c", hh=h),
                            acc[:, 0:hc].rearrange("p (hh c) -> p hh c", hh=h),
                            rec[:].unsqueeze(-1).to_broadcast([P, h, HID]),
                            mybir.AluOpType.mult)
                        nc.vector.tensor_tensor(xb[:], xb[:], biases[L][:, 0:hc],
                                                mybir.AluOpType.add)
                        # ELU = max(x,0) + exp(min(x,0)) - 1
                        xmin = fpool.tile([P, hc], f32, tag="xmin")
                        nc.vector.tensor_scalar_min(xmin[:], xb[:], 0.0)
                        em = fpool.tile([P, hc], f32, tag="em")
                        nc.scalar.activation(em[:], xmin[:],
                                             mybir.ActivationFunctionType.Exp)
                        xmax = fpool.tile([P, hc], f32, tag="xmax")
                        nc.vector.tensor_scalar_max(xmax[:], xb[:], 0.0)
                        if L < 2:
                            x2b = fpool.tile([P, hc], f32, tag="x2b")
                            nc.vector.scalar_tensor_tensor(
                                x2b[:], em[:], -1.0, xmax[:],
                                mybir.AluOpType.add, mybir.AluOpType.add)
                            for cchunk in range(2):
                                pt = psmisc.tile([P, P], f32, space="PSUM",
                                                 tag="ms", name="pt")
                                nc.tensor.transpose(
                                    pt[:], x2b[:, cchunk * P:(cchunk + 1) * P],
                                    ident[:])
                                nc.vector.tensor_copy(
                                    xT[:, cchunk, b * P:(b + 1) * P], pt[:])
                        else:
                            x4 = fpool.tile([P, HID], f32r, tag="x4")
                            nc.vector.scalar_tensor_tensor(
                                x4[:], em[:], -1.0, xmax[:],
                                mybir.AluOpType.add, mybir.AluOpType.add)
                            bsel = fpool.tile([P, NG], f32r, tag="bsel")
                            nc.vector.tensor_tensor(
                                bsel[:],
                                batc[:, b:b + 1].to_broadcast([P, NG]),
                                iota2[:, 0:NG], mybir.AluOpType.is_equal)
                            nc.tensor.matmul(
                                pool_ps[:], bsel[:], x4[:],
                                start=(b == 0), stop=(b == NB - 1),
                                skip_group_check=True)
                    ti += tg

            reps = max(1, int(time_reps))

            def repeat(fn):
                if reps == 1:
                    fn()
                else:
                    with tc.For_i(0, reps, 1):
                        fn()

            for L in range(3):
                if f"d{L + 1}" in stages:
                    repeat(lambda L=L: dense_phase(L))
                    nc.gpsimd.collective_compute(
                        "AllGather", mybir.AluOpType.bypass,
                        replica_groups=[list(range(NCORES))],
                        ins=[tslice[L].opt()], outs=[tfull[L].opt()])
                if f"e{L + 1}" in stages:
                    repeat(lambda L=L: edge_phase(L))

            if "pool" not in stages:
                # debug dumps: first table-slice block + xT slice
                dts = wpool.tile([P, EWM12], f32, tag="dts")
                nc.sync.dma_start(dts[:], tslice[0][0:P, 0:EWM12])
                nc.sync.dma_start(t_dbg1.ap(), dts[:])
                dxt = wpool.tile([P, 2 * P], f32, tag="dxt")
                nc.vector.tensor_copy(dxt[:, 0:P], xT[:, 0, 0:P].bitcast(f32))
                nc.vector.tensor_copy(dxt[:, P:2 * P], xT[:, 1, 0:P].bitcast(f32))
                nc.sync.dma_start(t_dbg2.ap(), dxt[:])
                fin0 = wpool.tile([NG, OUT], f32, tag="finout")
                nc.vector.memset(fin0[:], 0.0)
                nc.sync.dma_start(t_out.ap(), fin0[:])
            else:
                # pooling tail
                pp = wpool.tile([NG, HID], f32, tag="pp")
                nc.vector.tensor_copy(pp[:], pool_ps[:])
                nc.sync.dma_start(pool_in[:], pp[:])
                nc.gpsimd.collective_compute(
                    "AllReduce", mybir.AluOpType.add,
                    replica_groups=[list(range(NCORES))],
                    ins=[pool_in.opt()], outs=[pool_out.opt()])
                pooled = wpool.tile([NG, HID], f32, tag="pooled")
                nc.sync.dma_start(pooled[:], pool_out[:])
                pscal = wpool.tile([NG, HID], f32, tag="pscal")
                nc.scalar.activation(pscal[:], pooled[:],
                                     mybir.ActivationFunctionType.Copy,
                                     scale=icnt[:])
                ptp = psmisc.tile([NG, NG], f32, space="PSUM", tag="ms",
                                  name="ptp")
                nc.tensor.transpose(ptp[:], pscal[:, 0:NG], ident[0:NG, 0:NG])
                zconst = wpool.tile([P, NG], f32, tag="zconst")
                nc.vector.memset(zconst[:], 0.0)
                pT = wpool.tile([P, NG], f32r, tag="pT")
                nc.vector.tensor_copy(pT[:], zconst[:])
                nc.vector.tensor_copy(pT[0:NG, :], ptp[:])
                ops = psmisc.tile([NG, OUT], f32, space="PSUM", tag="ms",
                                  name="ops")
                nc.tensor.matmul(ops[:], pT[:], linw[:], start=True, stop=True)
                fin = wpool.tile([NG, OUT], f32, tag="finout")
                nc.vector.tensor_tensor(fin[:], ops[:], linb[:],
                                        mybir.AluOpType.add)
                nc.sync.dma_start(t_out.ap(), fin[:])

    nc.compile()
    return nc


# ----------------------------------------------------------------------------
# host orchestration
# ----------------------------------------------------------------------------

def _prepare(inputs):
    x = np.asarray(inputs["x"], dtype=np.float32)
    ei = np.asarray(inputs["edge_index"])
    batch = np.asarray(inputs["batch"])
    loops = np.arange(N, dtype=np.int64)
    src = np.concatenate([ei[0].astype(np.int64), loops])
    dst = np.concatenate([ei[1].astype(np.int64), loops])

    perm = _assign_nodes(src, dst)
    srcrow = perm[src]
    dstrow = perm[dst]
    sched = _build_schedule(srcrow, dstrow)

    # inverse permutation: row -> original node (-1 for pad rows)
    row_node = np.full(NTOT, -1, dtype=np.int64)
    row_node[perm] = np.arange(N)

    w1a = _augment_weights(np.asarray(inputs["W1"], np.float32),
                           np.asarray(inputs["asrc1"], np.float32),
                           np.asarray(inputs["adst1"], np.float32), EWM12)
    w2a = _augment_weights(np.asarray(inputs["W2"], np.float32),
                           np.asarray(inputs["asrc2"], np.float32),
                           np.asarray(inputs["adst2"], np.float32), EWM12)
    w3a = _augment_weights(np.asarray(inputs["W3"], np.float32),
                           np.asarray(inputs["asrc3"], np.float32),
                           np.asarray(inputs["adst3"], np.float32), EWM3)

    iota = np.arange(P, dtype=np.float32)
    iota2 = np.tile(iota, (P, 2)).reshape(P, 2 * P)
    ident = np.eye(P, dtype=np.float32)

    cnts = np.bincount(batch, minlength=NG).astype(np.float32)
    invcnt = (1.0 / np.maximum(cnts, 1.0)).reshape(NG, 1)
    linb = np.tile(np.asarray(inputs["linb"], np.float32), (NG, 1))

    in_maps = []
    for c in range(NCORES):
        rows = row_node[c * NODES_PER:(c + 1) * NODES_PER]
        xT = np.zeros((P, NODES_PER), np.float32)
        valid = rows >= 0
        xT[:, valid] = x[rows[valid]].T
        batc = np.full((NB, P), -1.0, np.float32)
        bflat = batc.reshape(-1)
        bflat[valid] = batch[rows[valid]].astype(np.float32)
        in_maps.append({
            "xT": xT,
            "W1a": w1a,
            "W2a": np.stack([w2a[0:P], w2a[P:2 * P]], axis=1),
            "W3a": np.stack([w3a[0:P], w3a[P:2 * P]], axis=1),
            "BIAS1": np.tile(np.asarray(inputs["b1"], np.float32), (P, 1)),
            "BIAS2": np.tile(np.asarray(inputs["b2"], np.float32), (P, 1)),
            "BIAS3": np.tile(np.asarray(inputs["b3"], np.float32), (P, 1)),
            "IOTA2": iota2,
            "IDENT": ident,
            "HIDX": sched["hidx"][c],
            "AIDX": sched["aidx"][c],
            "DLOC": sched["dloc"][c],
            "BATCH": batc.T.copy(),
            "INVCNT": invcnt,
            "LINW": np.concatenate([np.asarray(inputs["linW"], np.float32), np.zeros((P - HID, OUT), np.float32)], axis=0),
            "LINB": linb,
        })
    return sched, in_maps


def kernel(**inputs):
    sched, in_maps = _prepare(inputs)
    nc = _build_program(sched, time_reps=1)
    res = bass_utils.run_bass_kernel_spmd(nc, in_maps, core_ids=list(range(NCORES)))
    return res.results[0]["out"].astype(np.float32)

